# revision 2
# baseline (speedup 1.0000x reference)
"""ConvCNP1d Trainium2 kernel, v2: banded RBF via host-side sorting.

ls = ln2 over a 128-unit data range means exp(-0.5 d^2/ls^2) < e^-7.5
beyond |d| ~ 2.7 units, so both kernel matrices are banded once xc / xt
are sorted (host-side; the output is un-sorted at the end).

Encoder (per batch): the t-grid is cut into 8 value-blocks of 256 points.
For block k the xc's inside [t_lo - m, t_hi + m] are gathered (host) into
NCH_E[k] chunks of 128 sorted points (NCH_E = max over batches, so the
single SPMD program fits every core).  Per chunk one scalar_tensor_tensor
(DVE or Pool) writes a*t'^2 - 2a*xc'*t' into a PSUM tile [128, NCH*256];
one rank-2 PE matmul against a block-diagonal 0/1 table adds the
per-(chunk,partition) a*xc'^2 term (hi/lo fp16 split keeps it exact);
a single fused Exp activation emits the whole K tile in fp16 (one Scalar
op per block instead of per chunk); NCH accumulating matmuls against the
phi weights give the block's h [2, 256] in PSUM.

Decoder mirrors this with targets and grid swapped, but blocks are
xt *value*-blocks aligned to the grid so the window chunk indices are
compile-time constants shared by all cores: block k holds the sorted xt
falling in grid span [256k, 256(k+1)), padded to a uniform TGTU columns;
its window is the fixed grid chunks J0S[k] .. J0S[k]+NCH_D[k].  Each
block runs as two half-tiles of <=2 chunks so PSUM tiles stay <=2 banks.
Decoder [2, TGTU] outputs are DMA'd straight from PSUM to DRAM.

h0/h1 epilogue: reciprocal_approx_fast (single custom DVE op, ~18 bits).
Conv stack: 5 taps folded into the partition dim (shifted copies via
4x-mode DVE tensor_copy), so each layer is 4-8 matmuls, not 20-40.
conv4's identity/softplus epilogue runs in transposed [128, 16] layout
(PE transposes) which is also exactly the decoder's fT weight layout.
All activation functions used (Exp, Ln, Relu, Abs, Identity, Copy) are
grouped to minimize act-table reloads.
"""

import numpy as np

T_GRID = 2048
B = 16
N = 2048
NCORES = 8
BLOC = B // NCORES
NBLK = 8
WBLK = T_GRID // NBLK  # 256
ETH = 7.5              # exponent cutoff; kernel entries below e^-ETH dropped

_PROG_CACHE = {}


def build_program(cfg):
    import concourse.bacc as bacc
    import concourse.tile as tile
    from concourse import mybir

    f32 = mybir.dt.float32
    f16 = mybir.dt.float16
    AF = mybir.ActivationFunctionType
    OP = mybir.AluOpType
    import concourse.bass as bass_mod

    NCH_E = cfg["NCH_E"]
    NCH_D = cfg["NCH_D"]
    J0S = cfg["J0S"]
    TGTU = cfg["TGTU"]
    SE = sum(NCH_E)
    SD = sum(NCH_D)
    MAXNE = max(NCH_E)
    MAXND = max(NCH_D)
    os_rho = cfg["os_rho"]
    b4_0 = cfg["b4_0"]
    b4_1 = cfg["b4_1"]
    FRAC = cfg["stt_dve_frac"]
    assert MAXNE * WBLK <= 1024 and 2 * TGTU <= 1024

    nc = bacc.Bacc(None, target_bir_lowering=False)

    TE0h = nc.declare_dram_parameter("TE0", [1, WBLK], f32, isOutput=False)
    BDEh = nc.declare_dram_parameter("BDE", [2 * MAXNE + 2, MAXNE * WBLK], f16, isOutput=False)
    TCh = nc.declare_dram_parameter("TCONV", [16, T_GRID], f32, isOutput=False)
    W1h = nc.declare_dram_parameter("W1f", [2, 80], f16, isOutput=False)
    W2h = nc.declare_dram_parameter("W2f", [16, 160], f16, isOutput=False)
    W3h = nc.declare_dram_parameter("W3f", [128, 16], f16, isOutput=False)
    W3eh = nc.declare_dram_parameter("W3e", [32, 16], f16, isOutput=False)
    W4h = nc.declare_dram_parameter("W4f", [16, 10], f16, isOutput=False)
    B2h = nc.declare_dram_parameter("B2", [32, 1], f32, isOutput=False)
    B3h = nc.declare_dram_parameter("B3", [16, 1], f32, isOutput=False)
    ID2h = nc.declare_dram_parameter("ID2", [2, 2], f16, isOutput=False)
    XCSh = nc.declare_dram_parameter("XCS", [BLOC, 128, SE], f32, isOutput=False)
    PHIh = nc.declare_dram_parameter("PHI", [BLOC, 128, 2 * SE], f16, isOutput=False)
    XCBh = nc.declare_dram_parameter("XCB", [BLOC, 2 * MAXNE + 2, NBLK * 128], f16, isOutput=False)
    XTPh = nc.declare_dram_parameter("XTP", [BLOC, NBLK, TGTU], f32, isOutput=False)
    TDSh = nc.declare_dram_parameter("TDS", [BLOC, 128, SD], f32, isOutput=False)
    TDB0h = nc.declare_dram_parameter("TDB0", [BLOC, 6, NBLK * 128], f16, isOutput=False)
    TDB1h = nc.declare_dram_parameter("TDB1", [BLOC, 6, NBLK * 128], f16, isOutput=False)
    XTQ2h = nc.declare_dram_parameter("XTQ2", [BLOC, 6, NBLK * 2 * TGTU], f16, isOutput=False)
    OUTh = nc.declare_dram_parameter("out", [BLOC, 2, NBLK * TGTU], f32, isOutput=True)

    def bcast(dst, src_ap, n):
        nc.sync.dma_start(out=dst, in_=bass_mod.AP(
            tensor=src_ap.tensor, offset=src_ap.offset,
            ap=[[0, 128], [1, n]]))

    with tile.TileContext(nc) as tc:
        with (
            tc.tile_pool(name="singles", bufs=1) as singles,
            tc.tile_pool(name="perb", bufs=2) as perb,
            tc.tile_pool(name="kpool", bufs=3) as kpool,
            tc.tile_pool(name="k2keep", bufs=1) as k2keep,
            tc.tile_pool(name="small", bufs=1) as small,
            tc.tile_pool(name="psE", bufs=2, space="PSUM") as psE,
            tc.tile_pool(name="psC", bufs=2, space="PSUM") as psC,
            tc.tile_pool(name="psH", bufs=2, space="PSUM") as psH,
        ):
            TE0_sb = singles.tile([128, WBLK], f32)
            bcast(TE0_sb, TE0h[:, :], WBLK)
            BDE_sb = singles.tile([2 * MAXNE + 2, MAXNE * WBLK], f16)
            nc.sync.dma_start(out=BDE_sb, in_=BDEh[:, :])
            TC_sb = singles.tile([16, T_GRID], f32)
            nc.sync.dma_start(out=TC_sb, in_=TCh[:, :])
            W1_sb = singles.tile([2, 80], f16)
            nc.sync.dma_start(out=W1_sb, in_=W1h[:, :])
            W2_sb = singles.tile([16, 160], f16)
            nc.sync.dma_start(out=W2_sb, in_=W2h[:, :])
            W3_sb = singles.tile([128, 16], f16)
            nc.sync.dma_start(out=W3_sb, in_=W3h[:, :])
            W3e_sb = singles.tile([32, 16], f16)
            nc.sync.dma_start(out=W3e_sb, in_=W3eh[:, :])
            W4_sb = singles.tile([16, 10], f16)
            nc.sync.dma_start(out=W4_sb, in_=W4h[:, :])
            B2_sb = singles.tile([32, 1], f32)
            nc.sync.dma_start(out=B2_sb, in_=B2h[:, :])
            B3_sb = singles.tile([16, 1], f32)
            nc.sync.dma_start(out=B3_sb, in_=B3h[:, :])
            ID2_sb = singles.tile([2, 2], f16)
            nc.sync.dma_start(out=ID2_sb, in_=ID2h[:, :])

            st = [dict() for _ in range(BLOC)]

            def loads(b):
                s = st[b]
                s["XCS"] = perb.tile([128, SE], f32, tag="XCS", name="XCS_sb")
                nc.sync.dma_start(out=s["XCS"], in_=XCSh[b])
                s["PHI"] = perb.tile([128, 2 * SE], f16, tag="PHI", name="PHI_sb")
                nc.sync.dma_start(out=s["PHI"], in_=PHIh[b])
                s["XCB"] = perb.tile([2 * MAXNE + 2, NBLK * 128], f16, tag="XCB", name="XCB_sb")
                nc.sync.dma_start(out=s["XCB"], in_=XCBh[b])
                s["TDS"] = perb.tile([128, SD], f32, tag="TDS", name="TDS_sb")
                nc.sync.dma_start(out=s["TDS"], in_=TDSh[b])
                s["TDB0"] = perb.tile([6, NBLK * 128], f16, tag="TDB0", name="TDB0_sb")
                nc.sync.dma_start(out=s["TDB0"], in_=TDB0h[b])
                s["TDB1"] = perb.tile([6, NBLK * 128], f16, tag="TDB1", name="TDB1_sb")
                nc.sync.dma_start(out=s["TDB1"], in_=TDB1h[b])
                s["XTQ2"] = perb.tile([6, NBLK * 2 * TGTU], f16, tag="XTQ2", name="XTQ2_sb")
                nc.sync.dma_start(out=s["XTQ2"], in_=XTQ2h[b])
                for k in range(NBLK):
                    xp = perb.tile([128, TGTU], f32, tag=f"xtp{k}", name=f"xtp{k}")
                    bcast(xp, XTPh[b, k], TGTU)
                    s[f"xtp{k}"] = xp
                s["h"] = perb.tile([2, T_GRID], f32, tag="h_sb", name="h_sb")
                s["rep2"] = perb.tile([2, T_GRID + 4], f16, tag="rep2", name="rep2")
                nc.vector.memset(s["rep2"][:, 0:2], 0.0)
                nc.vector.memset(s["rep2"][:, T_GRID + 2:T_GRID + 4], 0.0)
                s["f1x5"] = perb.tile([16, T_GRID + 4], f16, tag="f1x5", name="f1x5")
                nc.vector.memset(s["f1x5"][0:16, 0:2], 0.0)
                nc.vector.memset(s["f1x5"][0:16, T_GRID + 2:T_GRID + 4], 0.0)
                s["f2x4"] = perb.tile([128, T_GRID + 4], f16, tag="f2x4", name="f2x4")
                nc.vector.memset(s["f2x4"][0:32, 0:2], 0.0)
                nc.vector.memset(s["f2x4"][0:32, T_GRID + 2:T_GRID + 4], 0.0)
                s["f3x5"] = perb.tile([16, T_GRID + 4], f16, tag="f3x5", name="f3x5")
                nc.vector.memset(s["f3x5"][0:16, 0:2], 0.0)
                nc.vector.memset(s["f3x5"][0:16, T_GRID + 2:T_GRID + 4], 0.0)
                s["fraw"] = perb.tile([2, T_GRID], f16, tag="fraw", name="fraw")
                s["fT"] = perb.tile([128, 2, 16], f16, tag="fT", name="fT")

            def kgen(in0, scal, base, nch, w, wb, bd, ktile, kslice0):
                """Rank-(2nch+2) PE matmul writes the per-(chunk,partition)
                bias plus the squared-coordinate row into PSUM (start=True);
                then per-chunk DVE STTs do E += in0*scal (RMW of the PE-
                written PSUM, the same pattern the conv TCONV add uses);
                one fused Exp emits the fp16 K tile."""
                tot = nch * w
                rows = 2 * nch + 2
                eps = psE.tile([128, 1024], f32, tag="E", name="E_ps")
                splits = ([(0, tot)] if tot <= 512
                          else [(0, 512), (512, tot)])
                for (c0, c1) in splits:
                    nc.tensor.matmul(
                        eps[:, c0:c1],
                        wb[0:rows, :],
                        bd[0:rows, c0:c1],
                        start=True, stop=True,
                    )
                for c in range(nch):
                    sl = slice(w * c, w * (c + 1))
                    nc.vector.scalar_tensor_tensor(
                        eps[:, sl],
                        in0, scal[:, base + c:base + c + 1], eps[:, sl],
                        OP.mult, OP.add,
                    )
                nc.scalar.activation(
                    out=ktile[:, kslice0:kslice0 + tot],
                    in_=eps[:, 0:tot], func=AF.Exp)

            def enc_block(b, k):
                s = st[b]
                nch = NCH_E[k]
                base = sum(NCH_E[:k])
                kt = kpool.tile([128, MAXNE * WBLK], f16, tag="K", name="K1t")
                kgen(TE0_sb, s["XCS"], base, nch, WBLK,
                     s["XCB"][:, 128 * k:128 * (k + 1)], BDE_sb, kt, 0)
                hps = psH.tile([2, WBLK], f32, tag="hms", name="h_ps")
                for c in range(nch):
                    nc.tensor.matmul(
                        hps,
                        s["PHI"][:, 2 * (base + c):2 * (base + c) + 2],
                        kt[:, WBLK * c:WBLK * (c + 1)],
                        start=(c == 0), stop=(c == nch - 1),
                    )
                nc.vector.tensor_copy(s["h"][:, WBLK * k:WBLK * (k + 1)], hps)

            def dec_half(b, k, half):
                s = st[b]
                nch = min(2, NCH_D[k] - 2 * half)
                base = sum(NCH_D[:k]) + 2 * half
                if half == 0:
                    s[f"k2t_{k}"] = k2keep.tile(
                        [128, MAXND * TGTU], f16, tag=f"k2_{b}_{k}",
                        name=f"k2_{b}_{k}")
                tdb = s["TDB0"] if half == 0 else s["TDB1"]
                kgen(s[f"xtp{k}"], s["TDS"], base, nch, TGTU,
                     tdb[:, 128 * k:128 * (k + 1)],
                     s["XTQ2"][:, 2 * TGTU * k:2 * TGTU * (k + 1)],
                     s[f"k2t_{k}"], 2 * half * TGTU)

            def epilogue(b):
                # row-1 reads/writes need DMA (compute engines are limited
                # to partition bases 0/32/64/96); latency is covered by the
                # other batch's encoder work in the emission order.
                s = st[b]
                rec = small.tile([1, T_GRID], f32, tag="rec", name="rec")
                h1 = small.tile([1, T_GRID], f32, tag="h1", name="h1")
                ratf = small.tile([1, T_GRID], f16, tag="ratf", name="ratf")
                nc.sync.dma_start(out=h1, in_=s["h"][1:2, :])
                nc.vector.reciprocal_approx_fast(rec, s["h"][0:1, :])
                nc.vector.tensor_mul(ratf, h1, rec)
                nc.sync.dma_start(out=s["rep2"][1:2, 2:2 + T_GRID], in_=ratf)
                nc.scalar.copy(s["rep2"][0:1, 2:2 + T_GRID], s["h"][0:1, :])

            def conv_layer(b, l):
                # taps fold into partitions only where the shifted copies
                # land on legal 32-aligned partition bases (conv3: 32ch);
                # conv1/2/4 run 5 accumulating tap-matmuls per chunk.
                s = st[b]
                if l == 0:
                    w_sb, it, O, taps = W1_sb, s["rep2"], 16, 5
                elif l == 1:
                    w_sb, it, O, taps = W2_sb, s["f1x5"], 32, 5
                elif l == 2:
                    for o in range(1, 4):
                        nc.vector.tensor_copy(
                            s["f2x4"][32 * o:32 * o + 32, 0:T_GRID + 4 - o],
                            s["f2x4"][0:32, o:T_GRID + 4])
                    w_sb, it, O, taps = W3_sb, s["f2x4"], 16, 0
                else:
                    w_sb, it, O, taps = W4_sb, s["f3x5"], 2, 5
                for n in range(4):
                    c0 = 512 * n
                    sl = slice(c0, c0 + 512)
                    ps = psC.tile([O, 512], f32, tag="c", name="c_ps")
                    if l == 2:
                        nc.tensor.matmul(ps, w_sb, it[:, sl],
                                         start=True, stop=False)
                        nc.tensor.matmul(ps, W3e_sb, it[0:32, c0 + 4:c0 + 516],
                                         start=False, stop=True)
                    else:
                        ni, no = {0: (2, 16), 1: (16, 32), 3: (16, 2)}[l]
                        for o in range(5):
                            nc.tensor.matmul(
                                ps, w_sb[:, no * o:no * (o + 1)],
                                it[0:ni, c0 + o:c0 + o + 512],
                                start=(o == 0), stop=(o == 4))
                    if l == 0:
                        nc.vector.tensor_add(ps, ps, TC_sb[:, sl])
                        nc.scalar.activation(
                            out=s["f1x5"][0:16, 2 + c0:2 + c0 + 512],
                            in_=ps, func=AF.Relu)
                    elif l == 1:
                        nc.scalar.activation(
                            out=s["f2x4"][0:32, 2 + c0:2 + c0 + 512],
                            in_=ps, func=AF.Relu, bias=B2_sb)
                    elif l == 2:
                        nc.scalar.activation(
                            out=s["f3x5"][0:16, 2 + c0:2 + c0 + 512],
                            in_=ps, func=AF.Relu, bias=B3_sb)
                    else:
                        nc.vector.tensor_copy(s["fraw"][:, sl], ps)

            def fchain(b):
                s = st[b]
                ftp = psC.tile([128, 32], f16, tag="c", name="ftp")
                for j in range(16):
                    nc.tensor.transpose(
                        ftp[:, 2 * j:2 * j + 2],
                        s["fraw"][:, 128 * j:128 * (j + 1)],
                        ID2_sb)
                t1 = small.tile([128, 16], f32, tag="t1", name="t1")
                t4 = small.tile([128, 16], f32, tag="t4", name="t4")
                nc.scalar.activation(
                    out=s["fT"][:, 0, :], in_=ftp[:, 0::2], func=AF.Identity,
                    scale=float(os_rho), bias=float(os_rho * b4_0))
                nc.scalar.activation(out=t1, in_=ftp[:, 1::2], func=AF.Abs,
                                     bias=float(b4_1))
                nc.scalar.activation(out=t1, in_=t1, func=AF.Exp, scale=-1.0)
                nc.scalar.activation(out=t1, in_=t1, func=AF.Ln, bias=1.0)
                nc.scalar.activation(out=t4, in_=ftp[:, 1::2], func=AF.Relu,
                                     scale=float(os_rho),
                                     bias=float(os_rho * b4_1))
                nc.vector.scalar_tensor_tensor(
                    s["fT"][:, 1, :], t1, float(os_rho), t4, OP.mult, OP.add)

            def dec_mm(b, k):
                s = st[b]
                kt = s[f"k2t_{k}"]
                nch = NCH_D[k]
                msps = psH.tile([2, TGTU], f32, tag="hms", name="ms_ps")
                for c in range(nch):
                    nc.tensor.matmul(
                        msps,
                        s["fT"][:, :, J0S[k] + c],
                        kt[:, TGTU * c:TGTU * (c + 1)],
                        start=(c == 0), stop=(c == nch - 1),
                    )
                osl = small.tile([2, TGTU], f32, tag="osb", name="osb",
                                 bufs=3)
                nc.vector.tensor_copy(osl, msps)
                nc.sync.dma_start(
                    out=OUTh[b, :, TGTU * k:TGTU * (k + 1)], in_=osl)

            # ---------------- emission ----------------
            loads(0)
            loads(1)
            for k in range(NBLK):
                enc_block(0, k)
            for k in range(NBLK):
                enc_block(1, k)
            epilogue(0)

            dec_units = [(b, k, h) for b in range(BLOC)
                         for k in range(NBLK)
                         for h in range(2) if 2 * h < NCH_D[k]]
            conv_units = [(0, 0), (1, 0), (0, 1), (1, 1),
                          (0, 2), (1, 2), (0, 3), (1, 3)]
            per = (len(dec_units) + len(conv_units) - 1) // len(conv_units)
            du = 0
            for i, (cb, cl) in enumerate(conv_units):
                if cb == 1 and cl == 0:
                    epilogue(1)
                conv_layer(cb, cl)
                for _ in range(per):
                    if du < len(dec_units):
                        b, k, h = dec_units[du]
                        dec_half(b, k, h)
                        du += 1
            while du < len(dec_units):
                b, k, h = dec_units[du]
                dec_half(b, k, h)
                du += 1

            fchain(0)
            for k in range(NBLK):
                dec_mm(0, k)
            fchain(1)
            for k in range(NBLK):
                dec_mm(1, k)

    nc.compile()
    return nc


def make_inmaps(inputs):
    f32 = np.float32
    f16 = np.float16
    f64 = np.float64
    xc = np.asarray(inputs["xc"])[..., 0].astype(f32)
    yc = np.asarray(inputs["yc"])[..., 0].astype(f32)
    xt = np.asarray(inputs["xt"])[..., 0].astype(f32)
    ls_psi = f64(np.float32(inputs["ls_psi"]))
    os_psi = f64(np.float32(inputs["os_psi"]))
    ls_rho = f64(np.float32(inputs["ls_rho"]))
    os_rho = f64(np.float32(inputs["os_rho"]))
    w = [np.asarray(inputs[f"w{i}"]).astype(f32) for i in (1, 2, 3, 4)]
    bs = [np.asarray(inputs[f"b{i}"]).astype(f32) for i in (1, 2, 3, 4)]

    lower = np.minimum(xc.min(), xt.min())
    upper = np.maximum(xc.max(), xt.max())
    t64 = np.linspace(f64(lower), f64(upper), T_GRID)
    delta = (t64[-1] - t64[0]) / (T_GRID - 1)

    a_psi = -0.5 / (ls_psi * ls_psi)
    a_rho = -0.5 / (ls_rho * ls_rho)
    m_psi = np.sqrt(ETH / -a_psi)
    m_rho = np.sqrt(ETH / -a_rho)
    MPTS = int(np.ceil(m_rho / delta))

    perm_c = np.argsort(xc, axis=1, kind="stable")
    xcs = np.take_along_axis(xc, perm_c, 1).astype(f64)
    ycs = np.take_along_axis(yc, perm_c, 1).astype(f64)
    perm_t = np.argsort(xt, axis=1, kind="stable")
    xts = np.take_along_axis(xt, perm_t, 1).astype(f64)

    # encoder windows
    eidx = np.zeros((B, NBLK, 2), np.int64)
    for k in range(NBLK):
        lo = t64[WBLK * k] - m_psi
        hi = t64[WBLK * (k + 1) - 1] + m_psi
        for b in range(B):
            eidx[b, k, 0] = np.searchsorted(xcs[b], lo)
            eidx[b, k, 1] = np.searchsorted(xcs[b], hi)
    ecnt = eidx[:, :, 1] - eidx[:, :, 0]
    NCH_E = [max(1, int(np.ceil(ecnt[:, k].max() / 128))) for k in range(NBLK)]
    assert max(NCH_E) <= 4, NCH_E

    # decoder quantile-blocks: 256 sorted targets each; window chunks are
    # derived from the extreme quantiles over ALL batches so the single
    # program covers every core.
    TGTU = WBLK
    J0S, J1S = [], []
    for k in range(NBLK):
        xmin = min(xts[b, WBLK * k] for b in range(B))
        xmax = max(xts[b, WBLK * (k + 1) - 1] for b in range(B))
        g0 = max(0, int(np.searchsorted(t64, xmin - m_rho)) - 1)
        g1 = min(T_GRID - 1, int(np.searchsorted(t64, xmax + m_rho)))
        j0 = g0 // 128
        j1 = g1 // 128 + 1
        J0S.append(j0)
        J1S.append(j1)
    NCH_D = [J1S[k] - J0S[k] for k in range(NBLK)]
    assert max(NCH_D) <= 4, NCH_D
    tsplit = [np.arange(NBLK + 1) * WBLK for _ in range(B)]
    SE = sum(NCH_E)
    SD = sum(NCH_D)
    MAXNE = max(NCH_E)
    MAXND = max(NCH_D)

    tpr = (np.arange(WBLK) - (WBLK - 1) / 2.0) * delta
    TE0 = tpr.astype(f32)[None, :]
    TE1 = a_psi * tpr * tpr

    def hi_lo(vals):
        hi = np.round(vals * 4.0) / 4.0
        lo = vals - hi
        return hi.astype(f16), lo.astype(f16)

    # BDE rhs rows: [TE1_hi tiled, TE1_lo tiled, then diag-ones pairs]
    BDE = np.zeros((2 * MAXNE + 2, MAXNE * WBLK), f16)
    te1_hi, te1_lo = hi_lo(TE1)
    for c in range(MAXNE):
        BDE[0, WBLK * c:WBLK * (c + 1)] = te1_hi
        BDE[1, WBLK * c:WBLK * (c + 1)] = te1_lo
        BDE[2 + 2 * c:4 + 2 * c, WBLK * c:WBLK * (c + 1)] = 1

    t_pad = np.zeros(T_GRID + 4, f64)
    t_pad[2:2 + T_GRID] = t64
    TCONV = np.zeros((16, T_GRID), f64)
    for o in range(5):
        TCONV += w[0][:, 0, o].astype(f64)[:, None] * t_pad[o:o + T_GRID][None, :]
    TCONV += bs[0].astype(f64)[:, None]

    W1f = np.zeros((2, 80), f16)      # [in=2, taps x out16]
    for o in range(5):
        W1f[:, 16 * o:16 * (o + 1)] = w[0][:, 1:3, o].T.astype(f16)
    W2f = np.zeros((16, 160), f16)
    for o in range(5):
        W2f[:, 32 * o:32 * (o + 1)] = w[1][:, :, o].T.astype(f16)
    W3f = np.zeros((128, 16), f16)    # taps 0-3 folded into partitions
    for o in range(4):
        W3f[32 * o:32 * (o + 1), :] = w[2][:, :, o].T.astype(f16)
    W3e = np.ascontiguousarray(w[2][:, :, 4].T).astype(f16)
    W4f = np.zeros((16, 10), f16)
    for o in range(5):
        W4f[:, 2 * o:2 * (o + 1)] = w[3][:, :, o].T.astype(f16)

    shared = {
        "TE0": TE0, "BDE": BDE,
        "TCONV": TCONV.astype(f32),
        "W1f": W1f, "W2f": W2f, "W3f": W3f, "W3e": W3e, "W4f": W4f,
        "B2": bs[1][:, None].copy(), "B3": bs[2][:, None].copy(),
        "ID2": np.eye(2, dtype=f16),
    }

    in_maps = []
    for core in range(NCORES):
        m = dict(shared)
        XCS = np.zeros((BLOC, 128, SE), f32)
        PHI = np.zeros((BLOC, 128, 2 * SE), f16)
        XCB = np.zeros((BLOC, 2 * MAXNE + 2, NBLK * 128), f16)
        XCB[:, 0:2, :] = 1
        XTP = np.zeros((BLOC, NBLK, TGTU), f32)
        XTQ2 = np.zeros((BLOC, 6, NBLK * 2 * TGTU), f16)
        TDS = np.zeros((BLOC, 128, SD), f32)
        TDB0 = np.zeros((BLOC, 6, NBLK * 128), f16)
        TDB0[:, 0:2, :] = 1
        TDB1 = np.zeros((BLOC, 6, NBLK * 128), f16)
        TDB1[:, 0:2, :] = 1
        for bb in range(BLOC):
            b = core * BLOC + bb
            base = 0
            for k in range(NBLK):
                ck = (t64[WBLK * k] + t64[WBLK * (k + 1) - 1]) / 2.0
                i0, i1 = eidx[b, k]
                nv = int(i1 - i0)
                ns = 128 * NCH_E[k]
                xv = np.zeros(ns, f64)
                xv[:nv] = xcs[b, i0:i1] - ck
                bias = np.full(ns, -60.0, f64)
                bias[:nv] = a_psi * xv[:nv] * xv[:nv]
                ph = np.zeros((ns, 2), f64)
                ph[:nv, 0] = os_psi
                ph[:nv, 1] = os_psi * ycs[b, i0:i1]
                for c in range(NCH_E[k]):
                    sl = slice(128 * c, 128 * (c + 1))
                    XCS[bb, :, base + c] = (-2.0 * a_psi * xv[sl]).astype(f32)
                    PHI[bb, :, 2 * (base + c)] = ph[sl, 0].astype(f16)
                    PHI[bb, :, 2 * (base + c) + 1] = ph[sl, 1].astype(f16)
                    hi, lo = hi_lo(bias[sl])
                    XCB[bb, 2 + 2 * c, 128 * k:128 * (k + 1)] = hi
                    XCB[bb, 3 + 2 * c, 128 * k:128 * (k + 1)] = lo
                base += NCH_E[k]
            base = 0
            for k in range(NBLK):
                gv = t64[128 * J0S[k]:128 * J1S[k]]
                cb = (gv[0] + gv[-1]) / 2.0
                i0, i1 = WBLK * k, WBLK * (k + 1)
                # window coverage check (chunks must span the band)
                assert xts[b, i0] - m_rho >= gv[0] - delta or J0S[k] == 0
                assert xts[b, i1 - 1] + m_rho <= gv[-1] + delta                     or J1S[k] == 16
                xv = xts[b, i0:i1] - cb
                XTP[bb, k, :] = xv.astype(f32)
                xq_hi, xq_lo = hi_lo(a_rho * xv * xv)
                k0 = 2 * TGTU * k
                for cc in range(2):
                    XTQ2[bb, 0, k0 + TGTU * cc:k0 + TGTU * (cc + 1)] = xq_hi
                    XTQ2[bb, 1, k0 + TGTU * cc:k0 + TGTU * (cc + 1)] = xq_lo
                    XTQ2[bb, 2 + 2 * cc:4 + 2 * cc,
                         k0 + TGTU * cc:k0 + TGTU * (cc + 1)] = 1
                tv = gv - cb
                for c in range(NCH_D[k]):
                    sl = slice(128 * c, 128 * (c + 1))
                    TDS[bb, :, base + c] = (-2.0 * a_rho * tv[sl]).astype(f32)
                    hi, lo = hi_lo(a_rho * tv[sl] * tv[sl])
                    half, cl = divmod(c, 2)
                    dst = TDB0 if half == 0 else TDB1
                    dst[bb, 2 + 2 * cl, 128 * k:128 * (k + 1)] = hi
                    dst[bb, 3 + 2 * cl, 128 * k:128 * (k + 1)] = lo
                base += NCH_D[k]
        m["XCS"] = XCS
        m["PHI"] = PHI
        m["XCB"] = XCB
        m["XTP"] = XTP
        m["XTQ2"] = XTQ2
        m["TDS"] = TDS
        m["TDB0"] = TDB0
        m["TDB1"] = TDB1
        in_maps.append(m)

    cfg = {
        "NCH_E": NCH_E, "NCH_D": NCH_D, "J0S": J0S, "TGTU": TGTU,
        "os_rho": float(os_rho), "b4_0": float(bs[3][0]),
        "b4_1": float(bs[3][1]), "stt_dve_frac": 1.0,
    }
    aux = {"perm_t": perm_t, "tsplit": tsplit, "TGTU": TGTU}
    return in_maps, cfg, aux


def kernel(**inputs):
    from concourse.bass_utils import run_bass_kernel_spmd

    in_maps, cfg, aux = make_inmaps(inputs)
    key = (tuple(cfg["NCH_E"]), tuple(cfg["NCH_D"]), tuple(cfg["J0S"]),
           cfg["TGTU"], cfg["os_rho"], cfg["b4_0"], cfg["b4_1"])
    if key not in _PROG_CACHE:
        _PROG_CACHE[key] = build_program(cfg)
    nc = _PROG_CACHE[key]

    res = run_bass_kernel_spmd(nc, in_maps, core_ids=list(range(NCORES)))
    outs = [np.asarray(res.results[i]["out"]) for i in range(NCORES)]
    packed = np.concatenate(outs, 0)  # [B, 2, N] in sorted-xt order
    out = np.zeros((B, N, 2), np.float32)
    for b in range(B):
        out[b, aux["perm_t"][b], 0] = packed[b, 0]
        out[b, aux["perm_t"][b], 1] = packed[b, 1]
    return out
